# revision 1
# baseline (speedup 1.0000x reference)
"""Trainium2 Bass kernel for nn_CAM (DANet channel-attention module).

Per batch element b (one per NeuronCore, 8 cores data-parallel over B=8):
    xf = x[b].reshape(C, H*W)                       # [512, 4096]
    E = xf @ xf.T                                   # [512, 512] (symmetric)
    att = softmax(max_j(E) - E, axis=-1)            # inverted softmax
    out = gamma * (att @ xf) + x[b]

Kernel math (identical in exact arithmetic to the reference):
    c[i]    = min_j E[i, j]         (= column min by symmetry)
    W[j, i] = exp(c[i] - E[j, i])   (= numerator of att[i, j]; exponent <= 0)
    S[i]    = sum_j W[j, i]
    out[i]  = gamma * (1/S[i]) * sum_j W[j, i] * xf[j, :] + x[b][i, :]

Design notes:
  - fp16 matmul path: same 10-bit mantissa as tf32/f32r (end-to-end rel err
    ~1e-2 at gamma=1) but full PE rate for matmuls AND transposes, FWL
    weight loads, and 2x DVE/ACT copy modes. PSUM accumulation is fp32 and
    the residual path (gamma * attn + x) is fp32 end-to-end.
  - x is cast to fp16 once on load; no other full-size copies
  - c (row-min) broadcast to the free axis via 4 PE transposes of a
    stride-0 broadcast AP instead of a DRAM roundtrip
  - input and output DMA on the sync engine; scalar engine does casts,
    PSUM->SBUF copies and exp; Exp table preloaded at kernel start
  - narrow leading input chunks and trailing output chunks shrink the
    pipeline start/drain
  - mm2 is chn-outer and the reps>1 timing chain goes through ping-pong,
    chunk-granular DRAM buffers: successive reps overlap like a real
    streaming workload while every output byte still feeds the next rep

reps > 1 unrolls the whole computation serially inside one NEFF with a
true data chain between reps (used by test.py to measure steady-state
per-iteration device time).
"""

import numpy as np

import concourse.bass as bass
import concourse.mybir as mybir
import concourse.tile as tile
from concourse import bacc
from concourse.masks import make_identity

P = 128          # partitions
C = 512          # channels
HW = 4096        # spatial (64*64)
CB = C // P      # 4 channel blocks
KB = HW // P     # 32 spatial blocks
NW = 512         # matmul free-dim chunk
NCH = HW // NW   # 8 n-chunks

F32 = mybir.dt.float32
F16 = mybir.dt.float16
EXP = mybir.ActivationFunctionType.Exp
ALU = mybir.AluOpType
AX = mybir.AxisListType

# input chunk widths (sum HW): narrow leading chunks cut pipeline startup
CHUNKS_IN = (128, 128, 256) + (NW,) * 7
# output chunk widths: narrow trailing chunks cut the drain tail
CHUNKS_OUT = (NW,) * 7 + (256, 256)

# symmetry: matmuls compute only blocks (jb, ib) with ib >= jb
RS = (0, P, 2 * P, 3 * P)
SYM = ((1, 0), (2, 0), (2, 1), (3, 0), (3, 1), (3, 2))


def build_nc(reps: int = 1):
    nc = bacc.Bacc("TRN2", target_bir_lowering=False)
    x = nc.dram_tensor("x", [C, HW], F32, kind="ExternalInput")
    g = nc.dram_tensor("gamma", [1], F32, kind="ExternalInput")
    y = nc.dram_tensor("y", [C, HW], F32, kind="ExternalOutput")

    with tile.TileContext(nc) as tc:
        with (
            tc.tile_pool(name="xin", bufs=1) as xin_pool,
            tc.tile_pool(name="xh", bufs=1) as xh_pool,
            tc.tile_pool(name="xtr", bufs=4) as xtr_pool,
            tc.tile_pool(name="w", bufs=1) as w_pool,
            tc.tile_pool(name="small", bufs=1) as small,
            tc.tile_pool(name="outp", bufs=4) as outp,
            tc.tile_pool(name="dram", bufs=2, space="DRAM") as dramp,
            tc.tile_pool(name="acc", bufs=4, space="PSUM") as acc_pool,
            tc.tile_pool(name="ptr", bufs=2, space="PSUM") as ptr_pool,
            tc.tile_pool(name="pout", bufs=2, space="PSUM") as pout_pool,
        ):
            # ---- constants (hoisted out of the rep loop)
            ident_f = small.tile([P, P], F32)
            make_identity(nc, ident_f)
            ident_h = small.tile([P, P], F16)
            nc.scalar.copy(out=ident_h, in_=ident_f)
            ones_h = small.tile([P, 8], F16)
            nc.vector.memset(ones_h, 1.0)
            gamma_bc = small.tile([P, 1], F32)
            nc.gpsimd.dma_start(out=gamma_bc, in_=g[:].partition_broadcast(P))
            # preload the Exp table so the first real exp isn't stalled
            warm = small.tile([P, 1], F32)
            nc.scalar.activation(out=warm, in_=gamma_bc, func=EXP)

            xr = x.rearrange("(t p) n -> p t n", p=P)
            yr = y.rearrange("(t p) n -> p t n", p=P)

            # ping-pong DRAM chain for reps > 1, split into NW-wide column
            # tiles so a rep's input chunk only depends on the previous
            # rep's stores to that chunk.
            if reps > 1:
                ybrs = [
                    [
                        dramp.tile(
                            [C, NW], F32, tag=f"ybuf{i}_{ci}",
                            name=f"ybuf{i}_{ci}",
                        ).rearrange("(t p) n -> p t n", p=P)
                        for ci in range(NCH)
                    ]
                    for i in range(2)
                ]

            for _rep in range(reps):
                in_chunks = None if (reps == 1 or _rep == 0) else ybrs[(_rep + 1) % 2]
                out_chunks = None if (reps == 1 or _rep == reps - 1) else ybrs[_rep % 2]

                def in_ap(pos, cw):
                    if in_chunks is None:
                        return xr[:, :, pos:pos + cw]
                    ti, off = pos // NW, pos % NW
                    return in_chunks[ti][:, :, off:off + cw]

                def out_ap(pos, cw):
                    if out_chunks is None:
                        return yr[:, :, pos:pos + cw]
                    ti, off = pos // NW, pos % NW
                    return out_chunks[ti][:, :, off:off + cw]

                X = xin_pool.tile([P, CB, HW], F32, tag="x")
                Xh = xh_pool.tile([P, CB, HW], F16, tag="xh")
                W = w_pool.tile([P, CB, C], F16, tag="w")
                Wt = w_pool.tile([P, CB, C], F16, tag="wt")
                rowmin = small.tile([P, CB], F32, tag="rowmin")
                c_sb = small.tile([P, C], F32, tag="csb")
                invsg = small.tile([P, CB], F32, tag="invsg")
                blk = small.tile([P, len(SYM), P], F32, tag="blk")

                # E accumulator banks (held across the fused load/T/mm1 loop)
                pe_tiles = [
                    acc_pool.tile([P, C], F32, tag="acc", name=f"pe_{_jb}")
                    for _jb in range(CB)
                ]

                # ---- fused: load chunk -> fp16 cast -> transposes -> mm1
                def _mm1(k, xt_k):
                    for jb in range(CB):
                        nc.tensor.matmul(
                            pe_tiles[jb][:, RS[jb]:],
                            lhsT=xt_k[:, jb * P:(jb + 1) * P],
                            rhs=xt_k[:, RS[jb]:],
                            start=(k == 0),
                            stop=(k == KB - 1),
                        )

                pend = []
                pos = 0
                for ci, cw in enumerate(CHUNKS_IN):
                    nsl = slice(pos, pos + cw)
                    nc.sync.dma_start(out=X[:, :, nsl], in_=in_ap(pos, cw))
                    nc.vector.tensor_copy(out=Xh[:, :, nsl], in_=X[:, :, nsl])
                    for kk in range(cw // P):
                        k = pos // P + kk
                        pxt = ptr_pool.tile([P, C], F16, tag="ptr", name="pxt")
                        for t in range(CB):
                            nc.tensor.transpose(
                                pxt[:, t * P:(t + 1) * P],
                                Xh[:, t, k * P:(k + 1) * P],
                                ident_h,
                            )
                        xt_k = xtr_pool.tile([P, C], F16, tag="xt")
                        if k % 2 == 0:
                            nc.vector.tensor_copy(out=xt_k, in_=pxt)
                        else:
                            nc.scalar.copy(out=xt_k, in_=pxt)
                        # software-pipelined emission: this k's matmuls are
                        # issued one step later so the PE transposes the next
                        # block instead of stalling on the PSUM->SBUF copy
                        pend.append((k, xt_k))
                        if len(pend) > 2:
                            _mm1(*pend.pop(0))
                    pos += cw

                while pend:
                    _mm1(*pend.pop(0))

                # ---- reconstruct the lower block-triangle (E[i,j]=E[j,i]^T)
                # and take row minima as soon as each row-block is complete
                def _rowmin(jb):
                    nc.vector.tensor_reduce(
                        out=rowmin[:, jb:jb + 1], in_=pe_tiles[jb],
                        axis=AX.X, op=ALU.min,
                    )

                _rowmin(0)
                for n6, (bi, bj) in enumerate(SYM):
                    nc.scalar.copy(
                        out=blk[:, n6, :], in_=pe_tiles[bj][:, bi * P:(bi + 1) * P]
                    )
                    nc.tensor.transpose(
                        pe_tiles[bi][:, bj * P:(bj + 1) * P], blk[:, n6, :], ident_f
                    )
                    if n6 == 0:
                        _rowmin(1)
                    elif n6 == 2:
                        _rowmin(2)
                    elif n6 == 5:
                        _rowmin(3)

                # ---- c to free-axis layout: transpose a stride-0 broadcast
                # of each rowmin column; block t of the result holds c[t*P+q]
                # replicated down all partitions.
                c_bc = ptr_pool.tile([P, C], F32, tag="ptr", name="cbc")
                for t in range(CB):
                    nc.tensor.transpose(
                        c_bc[:, t * P:(t + 1) * P],
                        rowmin[:, t:t + 1].broadcast_to([P, P]),
                        ident_f,
                    )
                nc.vector.tensor_copy(out=c_sb, in_=c_bc)

                # ---- W = exp(c - E)  ([j_part, i_free], fp16)
                for jb in range(CB):
                    nc.vector.tensor_tensor(
                        out=Wt[:, jb, :], in0=c_sb, in1=pe_tiles[jb],
                        op=ALU.subtract,
                    )
                    nc.scalar.activation(
                        out=W[:, jb, :], in_=Wt[:, jb, :], func=EXP
                    )

                # ---- S_i = sum_j W[j, i]; invsg = gamma / S
                S_ps = ptr_pool.tile([P, CB, 8], F32, tag="ptr", name="sps")
                for ib in range(CB):
                    for jb in range(CB):
                        nc.tensor.matmul(
                            S_ps[:, ib, :],
                            lhsT=W[:, jb, ib * P:(ib + 1) * P],
                            rhs=ones_h,
                            start=(jb == 0),
                            stop=(jb == CB - 1),
                        )
                nc.vector.reciprocal(out=invsg, in_=S_ps[:, :, 0])
                for ib in range(CB):
                    nc.vector.tensor_tensor(
                        out=invsg[:, ib:ib + 1], in0=invsg[:, ib:ib + 1],
                        in1=gamma_bc, op=ALU.mult,
                    )

                # ---- phase 2: out = invsg_i * (W^T @ xf) + x, chn-outer
                pos = 0
                for cw in CHUNKS_OUT:
                    nsl = slice(pos, pos + cw)
                    for ib in range(CB):
                        isl = slice(ib * P, (ib + 1) * P)
                        po_t = pout_pool.tile([P, cw], F32, tag="po", name="po")
                        for jb in range(CB):
                            nc.tensor.matmul(
                                po_t,
                                lhsT=W[:, jb, isl],
                                rhs=Xh[:, jb, nsl],
                                start=(jb == 0),
                                stop=(jb == CB - 1),
                            )
                        out_sb = outp.tile([P, cw], F32, tag="osb")
                        nc.vector.scalar_tensor_tensor(
                            out=out_sb,
                            in0=po_t,
                            scalar=invsg[:, ib:ib + 1],
                            in1=X[:, ib, nsl],
                            op0=ALU.mult,
                            op1=ALU.add,
                        )
                        st_eng = nc.scalar if ib % 2 == 0 else nc.sync
                        st_eng.dma_start(
                            out=out_ap(pos, cw)[:, ib, :], in_=out_sb
                        )
                    pos += cw

    nc.compile()
    return nc


_NC_CACHE = None


def _get_nc():
    global _NC_CACHE
    if _NC_CACHE is None:
        _NC_CACHE = build_nc()
    return _NC_CACHE


def kernel(x, gamma):
    from concourse.bass_utils import run_bass_kernel_spmd

    x = np.ascontiguousarray(np.asarray(x, dtype=np.float32))
    B = x.shape[0]
    assert x.shape == (8, C, 64, 64), x.shape
    xf = x.reshape(B, C, HW)
    gamma = np.ascontiguousarray(np.asarray(gamma, dtype=np.float32)).reshape(1)

    nc = _get_nc()
    in_maps = [{"x": xf[b], "gamma": gamma} for b in range(B)]
    res = run_bass_kernel_spmd(nc, in_maps, core_ids=list(range(B)))
    out = np.stack([res.results[b]["y"] for b in range(B)], axis=0)
    return out.reshape(B, C, 64, 64).astype(np.float32)

